# revision 47
# baseline (speedup 1.0000x reference)
"""Single-head causal attention (B=4, T=2048, C=1024, H=64) on 8 TRN2 NeuronCores.

Sharding: each batch b is handled by the core pair (2b, 2b+1). Within a pair,
keys/values are split by interleaved 128-row key-tiles (core parity p owns
global key-tiles {2m+p}).  Every core computes q/k/v projections from its own
1024 x-columns (host-supplied pre-transposed bf16, own-tiles-first), swaps qT
halves with its pair partner over core-to-core remote DMA, then computes
causal scores^T, exp, and the wei@[v|1] partial sums for ALL queries against
ITS OWN keys.  The host adds the two partial outputs of a pair and normalizes
(softmax denominator is the ones-column of the augmented v matmul).

Math notes:
 - scale = C**-0.5 = 1/32 folded into the exp activation's scale.
 - no max-subtraction: scores*scale ~ N(0, 0.25^2) so exp is tiny/safe.
 - compute in bf16 (fp32 PSUM accumulation); partial sums returned fp32.
 - stair masks are qt-independent in local column order: only 2 slot masks
   [128, 512] are needed ([TRI,1,c,1] and [0,TRI,0,c], c = parity coeff).
"""

import os
import sys

sys.path.insert(0, "/opt/trn_rl_repo")

import numpy as np
import ml_dtypes

B, T, C, H = 4, 2048, 1024, 64
NKT = 16  # global 128-row key tiles per batch
OWN = 8  # key tiles per core
QT = 4  # query tiles of 512 (in permuted local order)
SCALE = float(C) ** -0.5

_COMPILED = None
LAST_EXEC_NS = None
LAST_RESULTS = None


def _build_nc():
    """Hand-scheduled single-NEFF graph.  Engine queues execute in emission
    order, so the instruction streams are explicitly interleaved:

      PE : [qk + q-peer proj, interleaved per x-chunk (DMA-paced)]
           [v-proj n0] [v-proj n1] [sc j0 j1] [transp s0-3] [sc j2 j3]
           [transp s4-7] [sc j4 j5] [sc j6] [sc j7 j8] [pv qt3] [sc j9]
           [pv qt2] [pv qt1] [pv qt0]
      Act: input DMA issues, kT/vT psum->sbuf casts, then the 10 exps.
      DVE: qT strided casts, v_sb copies, stair-mask muls, pv->sbuf copies.

    Scores go through a ring of three 2-bank PSUM pools (A/B/C); score group
    j = (qt, slot pair) lands in pool j%3 and is exp'd as one [128,1024]
    activation.  qt runs DESCENDING so the drain tail is qt0 (2 slots).
    PSUM banks (8): C(2) + qk(2) + vq(2) during proj; qk/vq close (LIFO)
    and A/B take their banks; the 2 spare banks host the transpose pool,
    then the pv pool.  q-peer psum lives in partitions 64:128 of the v
    psum's banks (both are 64-partition tiles).
    """
    import concourse.bass as bass_mod
    import concourse.mybir as mybir
    import concourse.tile as tile
    from concourse import bacc
    from contextlib import ExitStack

    fp32 = mybir.dt.float32
    bf16 = mybir.dt.bfloat16

    nc = bacc.Bacc(
        "TRN2",
        target_bir_lowering=False,
        debug=False,
        num_devices=8,
        detect_race_conditions=True,
    )
    xT = nc.declare_dram_parameter("xT", [C, T], bf16, isOutput=False)
    wqk = nc.declare_dram_parameter("wqk", [128, 8 * 128], bf16, isOutput=False)
    wv = nc.declare_dram_parameter("wv", [128, 8 * H], bf16, isOutput=False)
    # Two slot masks [si, 128, 512] flattened (si=0 -> slot 2qt, si=1 -> 2qt+1)
    # ++ identity cols (PE transpose), so nothing needs the gpsimd ISA library.
    masks = nc.declare_dram_parameter("masks", [128, 2 * 512 + 128], bf16, isOutput=False)
    out_ext = nc.declare_dram_parameter("out", [H + 1, T], fp32, isOutput=True)

    with ExitStack() as ctx:
        tc = ctx.enter_context(tile.TileContext(nc))
        persist = ctx.enter_context(tc.tile_pool(name="persist", bufs=1))
        weipool = ctx.enter_context(tc.tile_pool(name="wei", bufs=4))

        # ---- loads (bf16 host-cast; HWDGE, spread over both queues).
        # x chunks split into column halves (a: 0:1024 feeds qk, b: 1024:2048
        # feeds q-peer) and issued in the order the PE consumes them, so the
        # first matmul can start as soon as c0a+wqk land (~9us).
        wqk_sb = persist.tile([128, 8, 128], bf16, tag="wqk_sb")
        wv_sb = persist.tile([128, 8, H], bf16, tag="wv_sb")
        xT_sb = persist.tile([128, 8, T], bf16, tag="xT_sb")

        def xload(eng, c):
            eng.dma_start(out=xT_sb[:, c, :], in_=xT[c * 128 : (c + 1) * 128, :])

        # wqk heads the scalar queue while x0 heads sync: per-queue transfers
        # run sequentially, so parallel queues land both ~1.3us sooner.
        nc.scalar.dma_start(out=wqk_sb[:], in_=wqk[:])
        xload(nc.sync, 0)
        xload(nc.scalar, 1)
        for c in (2, 4, 6):
            xload(nc.sync, c)
        for c in (3, 5, 7):
            xload(nc.scalar, c)
        nc.scalar.dma_start(out=wv_sb[:], in_=wv[:])
        mask_sb = persist.tile([128, 2 * 512 + 128], bf16, tag="mask_sb")
        nc.scalar.dma_start(out=mask_sb[:], in_=masks[:])
        ident = mask_sb[:, 1024:1152]

        qT_all = persist.tile([64, T], bf16, tag="qT_all")
        kT_own = persist.tile([64, 1024], bf16, tag="kT_own")
        vT_own = persist.tile([64, 1024], bf16, tag="vT_own")
        v_sb = persist.tile([128, 8, H + 1], bf16, tag="v_sb")
        nc.vector.memset(v_sb[:, :, H : H + 1], 1.0)
        out_sb = persist.tile([H + 1, T], fp32, tag="out_sb")

        def strided_pair_copy(dst_off, src_ap, n=2):
            """DVE copy of n 256-col chunks: src chunk i at 256*i, dst at
            512*i + dst_off (scatters psum q halves into qT_all layout)."""
            src = bass_mod.AP(
                tensor=src_ap.tensor,
                offset=src_ap.offset,
                ap=[src_ap.ap[0], [256, n], [1, 256]],
            )
            d = qT_all[:, dst_off : dst_off + 1]
            dst = bass_mod.AP(
                tensor=d.tensor, offset=d.offset, ap=[d.ap[0], [512, n], [1, 256]]
            )
            nc.vector.tensor_copy(dst, src)

        # ---- PSUM pools (creation order = bank placement; LIFO release) ----
        ps_rc = ctx.enter_context(tc.tile_pool(name="ps_rc", bufs=1, space="PSUM"))
        proj_ctx = ExitStack()
        ps_qk = proj_ctx.enter_context(tc.tile_pool(name="ps_qk", bufs=1, space="PSUM"))
        ps_vq = proj_ctx.enter_context(tc.tile_pool(name="ps_vq", bufs=1, space="PSUM"))
        qk_ps = ps_qk.tile([128, 1024], fp32, tag="qk", name="qk_ps")
        vq_ps = ps_vq.tile([128, 1024], fp32, tag="vq", name="vq_ps")
        vo_ps = vq_ps[0:64, :]
        qp_ps = vq_ps[64:128, :]

        # PE warmup: a 1-col junk matmul gated only on wqk's arrival, so the
        # PE queue/pipeline is rolling before the first x chunk lands.  The
        # real c0 chain (start=True) overwrites the written psum column.
        nc.tensor.matmul(
            out=qk_ps[:, 0:1],
            lhsT=wqk_sb[:, 0, :],
            rhs=wqk_sb[:, 0, 0:1],
            start=True,
            stop=True,
            skip_group_check=True,
        )

        # ---- PE: qk + q-peer projections, interleaved per K-chunk ----
        for c in range(8):
            for n in range(2):
                nc.tensor.matmul(
                    out=qk_ps[:, n * 512 : (n + 1) * 512],
                    lhsT=wqk_sb[:, c, :],
                    rhs=xT_sb[:, c, n * 512 : (n + 1) * 512],
                    start=(c == 0),
                    stop=(c == 7),
                )
            for n in range(2):
                nc.tensor.matmul(
                    out=qp_ps[:, n * 512 : (n + 1) * 512],
                    lhsT=wqk_sb[:, c, 0:64],
                    rhs=xT_sb[:, c, 1024 + n * 512 : 1024 + (n + 1) * 512],
                    start=(c == 0),
                    stop=(c == 7),
                )
        # v projection (x chunks all resident by now)
        for n in range(2):
            for c in range(8):
                nc.tensor.matmul(
                    out=vo_ps[:, n * 512 : (n + 1) * 512],
                    lhsT=wv_sb[:, c, :],
                    rhs=xT_sb[:, c, n * 512 : (n + 1) * 512],
                    start=(c == 0),
                    stop=(c == 7),
                )

        # psum -> sbuf casts: kT/vT on Act, qT own+peer strided on DVE
        # (h=1 halves first: attention starts with qt3)
        nc.scalar.copy(kT_own[:, 0:512], qk_ps[64:128, 0:512])
        nc.scalar.copy(kT_own[:, 512:1024], qk_ps[64:128, 512:1024])
        for h in (1, 0):
            cols = slice(h * 512, (h + 1) * 512)
            strided_pair_copy((2 * h) * 512, qk_ps[0:64, cols])
            strided_pair_copy((2 * h) * 512 + 256, qp_ps[:, cols])
        nc.scalar.copy(vT_own[:, 0:512], vo_ps[:, 0:512])
        nc.scalar.copy(vT_own[:, 512:1024], vo_ps[:, 512:1024])
        proj_ctx.close()  # frees qk/vq banks (LIFO) -> ring A/B

        ps_ra = ctx.enter_context(tc.tile_pool(name="ps_ra", bufs=1, space="PSUM"))
        ps_rb = ctx.enter_context(tc.tile_pool(name="ps_rb", bufs=1, space="PSUM"))
        ring = [
            ps_ra.tile([128, 1024], fp32, tag="ra", name="ra"),
            ps_rb.tile([128, 1024], fp32, tag="rb", name="rb"),
            ps_rc.tile([128, 1024], fp32, tag="rc", name="rc"),
        ]

        # score group j = (qt, slot pair k -> slots 2k,2k+1), ring pool j%3;
        # qt descending: j 0..3 = qt3, 4..6 = qt2, 7..8 = qt1, 9 = qt0
        glist = []
        j = 0
        for qt in (3, 2, 1, 0):
            for k in range(qt + 1):
                glist.append((qt, k, j % 3))
                j += 1
        wei_t = {
            qt: weipool.tile([128, 4096], bf16, tag="wei", name=f"wei{qt}")
            for qt in (3, 2, 1, 0)
        }

        def sc(i):
            qt, k, r = glist[i]
            for d in (0, 1):
                s = 2 * k + d
                nc.tensor.matmul(
                    out=ring[r][:, d * 512 : (d + 1) * 512],
                    lhsT=kT_own[:, s * 128 : (s + 1) * 128],
                    rhs=qT_all[:, qt * 512 : (qt + 1) * 512],
                    start=True,
                    stop=True,
                    skip_group_check=True,
                )

        def ex(i):
            qt, k, r = glist[i]
            nc.scalar.activation(
                out=wei_t[qt][:, 2 * k * 512 : (2 * k + 2) * 512],
                in_=ring[r][:, 0:1024],
                func=mybir.ActivationFunctionType.Exp,
                scale=SCALE,
            )

        def mask(qt):
            nc.vector.tensor_mul(
                out=wei_t[qt][:, 2 * qt * 512 : (2 * qt + 2) * 512],
                in0=wei_t[qt][:, 2 * qt * 512 : (2 * qt + 2) * 512],
                in1=mask_sb[:, 0:1024],
            )

        def pv_out(qt, ps_pv, split=False):
            nkt = 2 * qt + 2
            pv = ps_pv.tile([H + 1, 512], fp32, tag="pv")
            for s in range(nkt):
                nc.tensor.matmul(
                    out=pv[:],
                    lhsT=v_sb[:, s, :],
                    rhs=wei_t[qt][:, s * 512 : (s + 1) * 512],
                    start=(s == 0),
                    stop=(s == nkt - 1),
                    skip_group_check=True,
                )
            # split=True halves the copy+store so the first store's DMA setup
            # overlaps the second half's copy (trims the kernel drain tail)
            for lo, hi in ((0, 256), (256, 512)) if split else ((0, 512),):
                cols = slice(qt * 512 + lo, qt * 512 + hi)
                nc.vector.tensor_copy(out_sb[:, cols], pv[:, lo:hi])
                nc.sync.dma_start(out=out_ext[:, cols], in_=out_sb[:, cols])

        # ---- interleaved emission ----
        vt_ctx = ExitStack()
        ps_vt = vt_ctx.enter_context(tc.tile_pool(name="ps_vt", bufs=2, space="PSUM"))

        def transp(s):
            vt_ps = ps_vt.tile([128, H], bf16, tag="vt", name="vt_ps")
            nc.tensor.transpose(
                vt_ps[:], vT_own[:, s * 128 : (s + 1) * 128], ident[0:64, 0:64]
            )
            nc.vector.tensor_copy(v_sb[:, s, 0:H], vt_ps[:])

        sc(0); sc(1); ex(0); ex(1)
        for s in range(4):
            transp(s)
        sc(2); sc(3); ex(2); ex(3); mask(3)
        for s in range(4, 8):
            transp(s)
        sc(4); sc(5); ex(4); ex(5)
        sc(6); ex(6); mask(2)
        sc(7); sc(8); ex(7); ex(8); mask(1)
        sc(9); ex(9); mask(0)
        vt_ctx.close()
        ps_pv = ctx.enter_context(tc.tile_pool(name="ps_pv", bufs=2, space="PSUM"))
        pv_out(3, ps_pv)
        pv_out(2, ps_pv)
        pv_out(1, ps_pv)
        pv_out(0, ps_pv)

    nc.compile()
    return nc


def _local_q_perm(p):
    """global query index for each local column (length T)."""
    perm = np.empty(T, dtype=np.int64)
    for qt in range(QT):
        blk_kts = [4 * qt + p, 4 * qt + 2 + p, 4 * qt + (1 - p), 4 * qt + 2 + (1 - p)]
        for i, kt in enumerate(blk_kts):
            lo = qt * 512 + i * 128
            perm[lo : lo + 128] = np.arange(kt * 128, kt * 128 + 128)
    return perm


def _build_masks(p):
    """[128, 2*512] bf16: the two stair slot masks (qt-independent in local
    order): si=0 -> [TRI,1,c,1], si=1 -> [0,TRI,0,c], c = (p == 0)."""
    r = np.arange(128)
    j = np.arange(128)
    tri = (r[:, None] <= j[None, :]).astype(np.float32)
    one = np.ones((128, 128), dtype=np.float32)
    zero = np.zeros((128, 128), dtype=np.float32)
    cblk = one if p == 0 else zero
    m0 = np.concatenate([tri, one, cblk, one], axis=1)
    m1 = np.concatenate([zero, tri, zero, cblk], axis=1)
    ident = np.eye(128, dtype=np.float32)
    return np.ascontiguousarray(np.concatenate([m0, m1, ident], axis=1)).astype(
        ml_dtypes.bfloat16
    )


def _make_in_maps(x, Wq, Wk, Wv):
    bf16 = ml_dtypes.bfloat16
    # [C, 128|H] -> SBUF layout [p=128, c=8, j]: row p holds chunk-c row c*128+p
    wqk = np.concatenate([Wq, Wk], axis=1).reshape(8, 128, 128)
    wqk = np.ascontiguousarray(wqk.transpose(1, 0, 2).reshape(128, 8 * 128)).astype(bf16)
    wv = Wv.reshape(8, 128, H)
    wv = np.ascontiguousarray(wv.transpose(1, 0, 2).reshape(128, 8 * H)).astype(bf16)
    in_maps = []
    for c in range(8):
        b, p = c // 2, c % 2
        own_kts = [2 * m + p for m in range(8)]
        peer_kts = [2 * m + (1 - p) for m in range(8)]
        kts = own_kts + peer_kts
        rows = np.concatenate([np.arange(kt * 128, kt * 128 + 128) for kt in kts])
        xT_perm = np.ascontiguousarray(x[b][rows].T.astype(bf16))  # [C, 1024 or T]
        in_maps.append({"xT": xT_perm, "wqk": wqk, "wv": wv, "masks": _build_masks(p)})
    return in_maps


def _combine(per_core_out):
    """per_core_out: list of 8 arrays [H+1, T] (local query order) -> [B,T,H]."""
    out = np.empty((B, T, H), dtype=np.float32)
    for b in range(B):
        S = None
        for p in range(2):
            P_local = np.asarray(per_core_out[2 * b + p], dtype=np.float32)
            perm = _local_q_perm(p)
            P_glob = np.empty_like(P_local)
            P_glob[:, perm] = P_local
            S = P_glob if S is None else S + P_glob
        out[b] = (S[0:H, :] / S[H : H + 1, :]).T
    return out




def kernel(x, Wq, Wk, Wv):
    global _COMPILED, LAST_EXEC_NS, LAST_RESULTS
    from concourse.bass_utils import run_bass_kernel_spmd

    x = np.ascontiguousarray(np.asarray(x, dtype=np.float32))
    Wq = np.asarray(Wq, dtype=np.float32)
    Wk = np.asarray(Wk, dtype=np.float32)
    Wv = np.asarray(Wv, dtype=np.float32)

    if _COMPILED is None:
        _COMPILED = _build_nc()
    nc = _COMPILED

    in_maps = _make_in_maps(x, Wq, Wk, Wv)
    trace = os.environ.get("BASS_KERNEL_TRACE", "0") == "1"
    res = run_bass_kernel_spmd(nc, in_maps, core_ids=list(range(8)), trace=trace)
    LAST_EXEC_NS = getattr(res, "exec_time_ns", None)
    LAST_RESULTS = res
    return _combine([res.results[c]["out"] for c in range(8)])
